# revision 22
# baseline (speedup 1.0000x reference)
"""Trainium2 Bass kernel for AttentionWithRelPos.

Reference computation (fp32):
    qkv = x @ w_qkv.T                      # [B, N, 3C]
    q, k, v = split/reshape                # [B, H, N, HD]
    attn = softmax(q @ k.T * scale + bias) # bias gathered from rel_pos
    out  = (attn @ v).merge_heads @ w_proj.T + b_proj

Sharding: data-parallel over batch across 8 NeuronCores (8 batches/core).
All matmuls in bf16 with fp32 PSUM accumulation. Softmax is max-subtracted
(numerically safe for any input scale).

Per-core device pipeline (all feature-major / transposed layouts chosen so
no device-side transposes are needed except the softmax matrix itself):
  1. qkT = WqkT.T-stationary @ xT            -> [1536, 1576]   (q rows scaled)
  2. v   = xT-stationary @ WvT               -> [1576, 768]  (per-batch k-tiles)
  3. per (b, h):  S = qT.T @ kT  (q on partitions, k free)
     t = S + bias ; m = rowmax(t)  (fused DVE tensor_tensor_reduce)
     P = exp(t - m), rowsum via ACT accum_out ; r = 1/rowsum
     Pn = P * r (normalized, bf16)
     PnT = PE-transpose(Pn)  (4 blocks of <=128x128)
     outT = v-slice.T-stationary @ PnT       -> [64, 197] = attn-out head rows
  4. y = attT.T-stationary @ WpT             -> [1576, 768], PSUM -> DRAM
Host adds b_proj and re-assembles [64, 197, 768].
"""

import sys

if "/opt/trn_rl_repo" not in sys.path:
    sys.path.insert(0, "/opt/trn_rl_repo")

import numpy as np
import ml_dtypes

BF16 = ml_dtypes.bfloat16

B, DIM, HEADS, N = 64, 768, 12, 197
HD = DIM // HEADS  # 64
SCALE = HD ** -0.5
NCORES = 8
BL = B // NCORES  # 8 batches per core
KC = DIM // 128  # 6 contraction chunks

_CACHE = {}
BIAS_F32 = False
USE_TTR = False


def _build(bl=BL, probe=4, bias_f32=False):
    """Build + compile the per-core Bass program. Returns the compiled nc.

    probe: debug level — 0 skips attention; 1 up to S+ttr; 2 +exp/pn;
    3 +transposes; 4 full.
    """
    import concourse.bacc as bacc
    import concourse.bass as bass
    import concourse.tile as tile
    from concourse import mybir
    from contextlib import ExitStack

    sub = ""
    if isinstance(probe, str):
        probe, sub = 1, probe

    f32 = mybir.dt.float32
    bf16 = mybir.dt.bfloat16
    ALU = mybir.AluOpType
    ACTF = mybir.ActivationFunctionType

    tok = bl * N

    nc = bacc.Bacc("TRN2", target_bir_lowering=False, debug=False,
                   enable_asserts=False, num_devices=NCORES)

    xT = nc.dram_tensor("xT", (DIM, tok), bf16, kind="ExternalInput").ap()
    wqkT = nc.dram_tensor("wqkT", (DIM, 2 * DIM), bf16, kind="ExternalInput").ap()
    wvT = nc.dram_tensor("wvT", (DIM, DIM), bf16, kind="ExternalInput").ap()
    wpT = nc.dram_tensor("wpT", (DIM, DIM), bf16, kind="ExternalInput").ap()
    bias = nc.dram_tensor("bias", (HEADS, N, N), f32 if bias_f32 else bf16,
                          kind="ExternalInput").ap()
    ident = nc.dram_tensor("ident", (128, 128), bf16, kind="ExternalInput").ap()
    y = nc.dram_tensor("y", (tok, DIM), f32, kind="ExternalOutput").ap()

    # token-chunking for matmul moving dims
    NCH = 4 if tok % 4 == 0 else 1   # qk-proj rhs chunks
    CH = tok // NCH                  # 394 for bl=8
    assert CH <= 512
    # proj m-tiles (dense 128-token chunks)
    mt_sizes = [128] * (tok // 128) + ([tok % 128] if tok % 128 else [])

    with ExitStack() as ctx:
        tc = ctx.enter_context(tile.TileContext(nc))
        singles = ctx.enter_context(tc.tile_pool(name="singles", bufs=1))
        mm_psum = ctx.enter_context(tc.tile_pool(name="mm_psum", bufs=2, space="PSUM"))
        s_psum = ctx.enter_context(tc.tile_pool(name="s_psum", bufs=2, space="PSUM"))
        tr_psum = ctx.enter_context(tc.tile_pool(name="tr_psum", bufs=2, space="PSUM"))
        o_psum = ctx.enter_context(tc.tile_pool(name="o_psum", bufs=2, space="PSUM"))
        work = ctx.enter_context(tc.tile_pool(name="work", bufs=3))
        stats = ctx.enter_context(tc.tile_pool(name="stats", bufs=6))

        # ---- persistent SBUF tensors ----
        xT_sb = singles.tile([128, KC, tok], bf16)
        wqk_sb = singles.tile([128, KC, 2 * DIM], bf16)
        wv_sb = singles.tile([128, KC, DIM], bf16)
        wp_sb = singles.tile([128, KC, DIM], bf16)
        bias_sb = singles.tile([128, HEADS, 2, N], f32 if bias_f32 else bf16)
        id_sb = singles.tile([128, 128], bf16)
        qkT_sb = singles.tile([128, 2 * KC, tok], bf16)
        v_sb = singles.tile([128, bl, 2, DIM], bf16)
        attT_sb = singles.tile([128, KC, tok], bf16)

        # ---- input DMAs ----
        for kc in range(KC):
            nc.sync.dma_start(out=xT_sb[:, kc, :], in_=xT[kc * 128:(kc + 1) * 128, :])
            nc.sync.dma_start(out=wqk_sb[:, kc, :], in_=wqkT[kc * 128:(kc + 1) * 128, :])
            nc.sync.dma_start(out=wv_sb[:, kc, :], in_=wvT[kc * 128:(kc + 1) * 128, :])
            nc.sync.dma_start(out=wp_sb[:, kc, :], in_=wpT[kc * 128:(kc + 1) * 128, :])
        nc.sync.dma_start(out=id_sb[:, :], in_=ident[:, :])
        for h in range(HEADS):
            nc.sync.dma_start(out=bias_sb[:, h, 0, :], in_=bias[h, 0:128, :])
            nc.sync.dma_start(out=bias_sb[0:N - 128, h, 1, :], in_=bias[h, 128:N, :])

        # ---- stage 1: qkT = w_qk @ x.T  ([2*DIM, tok], feature-major) ----
        # m-order interleaves q-chunks and k-chunks so attention can start early.
        m_order = []
        for i in range(KC):
            m_order += [i, KC + i]
        for mi, m in enumerate(m_order):
            for n in range(NCH):
                ps = mm_psum.tile([128, 512], f32, tag="mm")
                for kc in range(KC):
                    nc.tensor.matmul(
                        ps[:, 0:CH],
                        lhsT=wqk_sb[:, kc, m * 128:(m + 1) * 128],
                        rhs=xT_sb[:, kc, n * CH:(n + 1) * CH],
                        start=(kc == 0), stop=(kc == KC - 1),
                    )
                dst = qkT_sb[:, m, n * CH:(n + 1) * CH]
                if (mi * NCH + n) % 2 == 0:
                    nc.scalar.copy(out=dst, in_=ps[:, 0:CH])
                else:
                    nc.vector.tensor_copy(dst, ps[:, 0:CH])

        # ---- stage 2: v = x @ w_v.T  (token-major, per-batch k-tiles) ----
        for b in range(bl):
            for kt in range(2):
                rows = 128 if kt == 0 else N - 128
                t0 = b * N + kt * 128
                for n2 in range(2):
                    ps = mm_psum.tile([128, 512], f32, tag="mm")
                    for kc in range(KC):
                        nc.tensor.matmul(
                            ps[0:rows, 0:384],
                            lhsT=xT_sb[:, kc, t0:t0 + rows],
                            rhs=wv_sb[:, kc, n2 * 384:(n2 + 1) * 384],
                            start=(kc == 0), stop=(kc == KC - 1),
                        )
                    dst = v_sb[0:rows, b, kt, n2 * 384:(n2 + 1) * 384]
                    if (b * 4 + kt * 2 + n2) % 2 == 0:
                        nc.scalar.copy(out=dst, in_=ps[0:rows, 0:384])
                    else:
                        nc.vector.tensor_copy(dst, ps[0:rows, 0:384])

        # ---- stage 3: attention per (b, h) ----
        qt_sizes = [128, N - 128]
        if probe < 4:
            nc.vector.memset(attT_sb[:, :, :], 0.0)
        for b in range(bl if probe >= 1 else 0):
            for h in range(HEADS):
                if sub == "s_even" and h % 2:
                    continue
                mq = h // 2
                mk = KC + h // 2
                po = (h % 2) * 64
                qT = qkT_sb[po:po + 64, mq, b * N:(b + 1) * N]
                kT = qkT_sb[po:po + 64, mk, b * N:(b + 1) * N]

                pn = (work.tile([128, 2, N], bf16, tag="pn", name="pn")
                      if probe >= 2 else None)
                rstat = []
                for qt in range(2):
                    qn = qt_sizes[qt]
                    s_ps = s_psum.tile([128, N], f32, tag="s", name="s_ps")
                    nc.tensor.matmul(
                        s_ps[0:qn, :],
                        lhsT=qT[:, qt * 128:qt * 128 + qn],
                        rhs=kT,
                        start=True, stop=True,
                    )
                    if sub in ("s_even", "s_all"):
                        continue
                    t_sb = work.tile([128, N], f32, tag="t", name="t_sb")
                    negm = stats.tile([128, 1], f32, tag="negm")
                    if USE_TTR:
                        mmax = stats.tile([128, 1], f32, tag="mmax")
                        nc.vector.tensor_tensor_reduce(
                            out=t_sb[0:qn, :],
                            in0=s_ps[0:qn, :],
                            in1=bias_sb[0:qn, h, qt, :],
                            scale=1.0,
                            scalar=-3.0e38,
                            op0=ALU.add,
                            op1=ALU.max,
                            accum_out=mmax[0:qn, :],
                        )
                        if sub == "ttr":
                            continue
                        nc.vector.tensor_scalar_mul(
                            negm[0:qn, :], mmax[0:qn, :], -1.0)
                    else:
                        nc.vector.tensor_add(
                            t_sb[0:qn, :], s_ps[0:qn, :], bias_sb[0:qn, h, qt, :])
                        if sub == "ttr":
                            continue
                        nc.vector.tensor_reduce(
                            out=negm[0:qn, :], in_=t_sb[0:qn, :],
                            axis=mybir.AxisListType.X, op=ALU.max, negate=True,
                        )
                    if probe < 2:
                        continue
                    p_sb = work.tile([128, N], f32, tag="p")
                    rsum = stats.tile([128, 1], f32, tag="rsum")
                    nc.scalar.activation(
                        out=p_sb[0:qn, :],
                        in_=t_sb[0:qn, :],
                        func=ACTF.Exp,
                        bias=negm[0:qn, :],
                        scale=1.0,
                        accum_out=rsum[0:qn, :],
                    )
                    rcp = stats.tile([128, 1], f32, tag="rcp")
                    nc.vector.reciprocal(rcp[0:qn, :], rsum[0:qn, :])
                    nc.vector.tensor_scalar_mul(
                        pn[0:qn, qt, :], p_sb[0:qn, :], rcp[0:qn, :]
                    )
                    rstat.append(rcp)

                if probe < 3:
                    continue
                # transpose Pn -> PnT  (4 blocks inside one PSUM bank)
                pnT = work.tile([128, 2, N], bf16, tag="pnT")
                tr = tr_psum.tile([128, 512], bf16, tag="tr")
                for kt in range(2):
                    kn = qt_sizes[kt]
                    for qt in range(2):
                        qn = qt_sizes[qt]
                        blk = tr[0:kn, (kt * 2 + qt) * 128:(kt * 2 + qt) * 128 + qn]
                        nc.tensor.transpose(
                            blk,
                            in_=pn[0:qn, qt, kt * 128:kt * 128 + kn],
                            identity=id_sb[0:qn, 0:qn],
                        )
                        nc.scalar.copy(
                            out=pnT[0:kn, kt, qt * 128:qt * 128 + qn],
                            in_=tr[0:kn, (kt * 2 + qt) * 128:(kt * 2 + qt) * 128 + qn],
                        )

                if probe < 4:
                    continue
                # PV: outT[d, q] accumulated over k-tiles
                o_ps = o_psum.tile([64, N], f32, tag="o")
                for kt in range(2):
                    kn = qt_sizes[kt]
                    nc.tensor.matmul(
                        o_ps[:, :],
                        lhsT=v_sb[0:kn, b, kt, h * 64:(h + 1) * 64],
                        rhs=pnT[0:kn, kt, :],
                        start=(kt == 0), stop=(kt == 1),
                    )
                nc.scalar.copy(
                    out=attT_sb[po:po + 64, mq, b * N:(b + 1) * N],
                    in_=o_ps[:, :],
                )

        # ---- stage 4: y = att @ w_proj.T  (token-major, PSUM -> DRAM) ----
        for mt, rows in enumerate(mt_sizes):
            t0 = mt * 128
            for n2 in range(2):
                ps = mm_psum.tile([128, 512], f32, tag="mm")
                for kc in range(KC):
                    nc.tensor.matmul(
                        ps[0:rows, 0:384],
                        lhsT=attT_sb[:, kc, t0:t0 + rows],
                        rhs=wp_sb[:, kc, n2 * 384:(n2 + 1) * 384],
                        start=(kc == 0), stop=(kc == KC - 1),
                    )
                yst = work.tile([128, 384], f32, tag="yst")
                if (mt * 2 + n2) % 2 == 0:
                    nc.scalar.copy(out=yst[0:rows, :], in_=ps[0:rows, 0:384])
                else:
                    nc.vector.tensor_copy(yst[0:rows, :], ps[0:rows, 0:384])
                nc.sync.dma_start(
                    out=y[t0:t0 + rows, n2 * 384:(n2 + 1) * 384],
                    in_=yst[0:rows, :],
                )

    nc.compile()
    return nc


def _prep_shared(w_qkv, w_proj, rel_pos, rel_pos_index):
    """Host-side input prep shared across cores (weights / bias / identity)."""
    w_qkv = np.asarray(w_qkv, dtype=np.float32)
    w_proj = np.asarray(w_proj, dtype=np.float32)
    rel_pos = np.asarray(rel_pos, dtype=np.float32)
    rel_pos_index = np.asarray(rel_pos_index)

    wqk = w_qkv[:2 * DIM].copy()
    wqk[:DIM] *= SCALE  # fold attention scale into Wq
    wqkT = np.ascontiguousarray(wqk.T).astype(BF16)
    wvT = np.ascontiguousarray(w_qkv[2 * DIM:].T).astype(BF16)
    wpT = np.ascontiguousarray(w_proj.T).astype(BF16)

    bias_full = np.zeros((HEADS, N, N), dtype=np.float32)
    bias_full[:, 1:, 1:] = rel_pos[:, rel_pos_index]
    bias_out = bias_full if BIAS_F32 else bias_full.astype(BF16)

    ident = np.eye(128, dtype=BF16)
    return {"wqkT": wqkT, "wvT": wvT, "wpT": wpT, "bias": bias_out, "ident": ident}


def _prep_core(x, core, bl=BL):
    """Per-core xT: [DIM, bl*N] bf16."""
    xc = np.asarray(x[core * bl:(core + 1) * bl], dtype=np.float32)
    xT = np.ascontiguousarray(xc.reshape(bl * N, DIM).T).astype(BF16)
    return xT


def kernel(x, w_qkv, w_proj, b_proj, rel_pos, rel_pos_index):
    from concourse.bass_utils import run_bass_kernel_spmd

    if "nc" not in _CACHE:
        _CACHE["nc"] = _build(BL)
    nc = _CACHE["nc"]

    shared = _prep_shared(w_qkv, w_proj, rel_pos, rel_pos_index)
    in_maps = []
    for core in range(NCORES):
        m = dict(shared)
        m["xT"] = _prep_core(x, core)
        in_maps.append(m)

    res = run_bass_kernel_spmd(nc, in_maps, core_ids=list(range(NCORES)))
    b_proj = np.asarray(b_proj, dtype=np.float32)
    y = np.concatenate(
        [r["y"].reshape(BL, N, DIM) for r in res.results], axis=0
    ).astype(np.float32)
    return y + b_proj[None, None, :]


# revision 37
# speedup vs baseline: 1.2662x; 1.2662x over previous
"""Trainium2 Bass kernel for AttentionWithRelPos.

Reference computation (fp32):
    qkv = x @ w_qkv.T                      # [B, N, 3C]
    q, k, v = split/reshape                # [B, H, N, HD]
    attn = softmax(q @ k.T * scale + bias) # bias gathered from rel_pos
    out  = (attn @ v).merge_heads @ w_proj.T + b_proj

Sharding: data-parallel over batch across 8 NeuronCores (8 batches/core).
All matmuls in bf16 with fp32 PSUM accumulation. Softmax is max-subtracted
(numerically safe for any input scale).

Per-core device pipeline (all feature-major / transposed layouts chosen so
no device-side transposes are needed except the softmax matrix itself):
  1. qkT = WqkT.T-stationary @ xT            -> [1536, 1576]   (q rows scaled)
  2. v   = xT-stationary @ WvT               -> [1576, 768]  (per-batch k-tiles)
  3. per (b, h):  S = qT.T @ kT  (q on partitions, k free)
     t = S + bias ; m = rowmax(t)  (fused DVE tensor_tensor_reduce)
     P = exp(t - m), rowsum via ACT accum_out ; r = 1/rowsum
     Pn = P * r (normalized, bf16)
     PnT = PE-transpose(Pn)  (4 blocks of <=128x128)
     outT = v-slice.T-stationary @ PnT       -> [64, 197] = attn-out head rows
  4. y = attT.T-stationary @ WpT             -> [1576, 768], PSUM -> DRAM
Host adds b_proj and re-assembles [64, 197, 768].
"""

import sys

if "/opt/trn_rl_repo" not in sys.path:
    sys.path.insert(0, "/opt/trn_rl_repo")

import numpy as np
import ml_dtypes

BF16 = ml_dtypes.bfloat16

B, DIM, HEADS, N = 64, 768, 12, 197
HD = DIM // HEADS  # 64
SCALE = HD ** -0.5
NCORES = 8
BL = B // NCORES  # 8 batches per core
KC = DIM // 128  # 6 contraction chunks

_CACHE = {}
BIAS_F32 = False
USE_TTR = False


def _build(bl=BL, probe=4, bias_f32=False):
    """Build + compile the per-core Bass program. Returns the compiled nc.

    probe: debug level — 0 skips attention; 1 up to S+ttr; 2 +exp/pn;
    3 +transposes; 4 full.
    """
    import concourse.bacc as bacc
    import concourse.bass as bass
    import concourse.tile as tile
    from concourse import mybir
    from contextlib import ExitStack

    sub = ""
    if isinstance(probe, str):
        probe, sub = 1, probe

    f32 = mybir.dt.float32
    bf16 = mybir.dt.bfloat16
    ALU = mybir.AluOpType
    ACTF = mybir.ActivationFunctionType

    tok = bl * N

    nc = bacc.Bacc("TRN2", target_bir_lowering=False, debug=False,
                   enable_asserts=False, num_devices=NCORES)

    xT = nc.dram_tensor("xT", (DIM, tok), bf16, kind="ExternalInput").ap()
    wqkT = nc.dram_tensor("wqkT", (DIM, 2 * DIM), bf16, kind="ExternalInput").ap()
    wvT = nc.dram_tensor("wvT", (DIM, DIM), bf16, kind="ExternalInput").ap()
    wpT = nc.dram_tensor("wpT", (DIM, DIM), bf16, kind="ExternalInput").ap()
    bias = nc.dram_tensor("bias", (HEADS, N, N), f32 if bias_f32 else bf16,
                          kind="ExternalInput").ap()
    ident = nc.dram_tensor("ident", (128, 128), bf16, kind="ExternalInput").ap()
    y = nc.dram_tensor("y", (tok, DIM), f32, kind="ExternalOutput").ap()

    # token-chunking for matmul moving dims
    NCH = 4 if tok % 4 == 0 else 1   # qk-proj rhs chunks
    CH = tok // NCH                  # 394 for bl=8
    assert CH <= 512
    # proj m-tiles (dense 128-token chunks)
    mt_sizes = [128] * (tok // 128) + ([tok % 128] if tok % 128 else [])

    with ExitStack() as ctx:
        tc = ctx.enter_context(tile.TileContext(nc))
        singles = ctx.enter_context(tc.tile_pool(name="singles", bufs=1))
        mm_psum = ctx.enter_context(tc.tile_pool(name="mm_psum", bufs=2, space="PSUM"))
        s_psum = ctx.enter_context(tc.tile_pool(name="s_psum", bufs=4, space="PSUM"))
        tr_psum = ctx.enter_context(tc.tile_pool(name="tr_psum", bufs=1, space="PSUM"))
        o_psum = ctx.enter_context(tc.tile_pool(name="o_psum", bufs=1, space="PSUM"))
        work = ctx.enter_context(tc.tile_pool(name="work", bufs=5))
        stats = ctx.enter_context(tc.tile_pool(name="stats", bufs=12))

        # ---- persistent SBUF tensors ----
        xT_sb = singles.tile([128, KC, tok], bf16)
        wqk_sb = singles.tile([128, KC, 2 * DIM], bf16)
        wv_sb = singles.tile([128, KC, DIM], bf16)
        wp_sb = singles.tile([128, KC, DIM], bf16)
        bias_sb = singles.tile([128, HEADS, 2, N], f32 if bias_f32 else bf16)
        id_sb = singles.tile([128, 128], bf16)
        qkT_sb = singles.tile([128, 2 * KC, tok], bf16)
        v_sb = singles.tile([128, bl, 2, DIM], bf16)
        attT_sb = singles.tile([128, KC, tok], bf16)

        # ---- input DMAs ----
        for kc in range(KC):
            nc.sync.dma_start(out=xT_sb[:, kc, :], in_=xT[kc * 128:(kc + 1) * 128, :])
            nc.sync.dma_start(out=wqk_sb[:, kc, :], in_=wqkT[kc * 128:(kc + 1) * 128, :])
            nc.sync.dma_start(out=wv_sb[:, kc, :], in_=wvT[kc * 128:(kc + 1) * 128, :])
            nc.sync.dma_start(out=wp_sb[:, kc, :], in_=wpT[kc * 128:(kc + 1) * 128, :])
        nc.sync.dma_start(out=id_sb[:, :], in_=ident[:, :])
        for h in range(HEADS):
            nc.sync.dma_start(out=bias_sb[:, h, 0, :], in_=bias[h, 0:128, :])
            nc.sync.dma_start(out=bias_sb[0:N - 128, h, 1, :], in_=bias[h, 128:N, :])

        qt_sizes = [128, N - 128]

        def emit_qkproj(m, mi):
            for n in range(NCH):
                ps = mm_psum.tile([128, 512], f32, tag="mm", name="ps")
                for kc in range(KC):
                    nc.tensor.matmul(
                        ps[:, 0:CH],
                        lhsT=wqk_sb[:, kc, m * 128:(m + 1) * 128],
                        rhs=xT_sb[:, kc, n * CH:(n + 1) * CH],
                        start=(kc == 0), stop=(kc == KC - 1),
                    )
                dst = qkT_sb[:, m, n * CH:(n + 1) * CH]
                nc.scalar.copy(out=dst, in_=ps[:, 0:CH])

        def emit_vproj(b):
            for kt in range(2):
                rows = 128 if kt == 0 else N - 128
                t0 = b * N + kt * 128
                for n2 in range(2):
                    ps = mm_psum.tile([128, 512], f32, tag="mm", name="ps")
                    for kc in range(KC):
                        nc.tensor.matmul(
                            ps[0:rows, 0:384],
                            lhsT=xT_sb[:, kc, t0:t0 + rows],
                            rhs=wv_sb[:, kc, n2 * 384:(n2 + 1) * 384],
                            start=(kc == 0), stop=(kc == KC - 1),
                        )
                    dst = v_sb[0:rows, b, kt, n2 * 384:(n2 + 1) * 384]
                    nc.vector.tensor_copy(dst, ps[0:rows, 0:384])

        def emit_attention(b, h):
            mq = h // 2
            mk = KC + h // 2
            po = (h % 2) * 64
            qT = qkT_sb[po:po + 64, mq, b * N:(b + 1) * N]
            kT = qkT_sb[po:po + 64, mk, b * N:(b + 1) * N]

            pn = work.tile([128, 2, N], bf16, tag="pn", name="pn")
            for qt in range(2):
                qn = qt_sizes[qt]
                s_ps = s_psum.tile([128, N], f32, tag="s", name="s_ps")
                # S = q.k^T; second matmul accumulates the rel-pos bias via
                # an identity-block stationary (bias rows are partition-major
                # in bias_sb)
                nc.tensor.matmul(
                    s_ps[0:qn, :],
                    lhsT=qT[:, qt * 128:qt * 128 + qn],
                    rhs=kT,
                    start=True, stop=False,
                )
                nc.tensor.matmul(
                    s_ps[0:qn, :],
                    lhsT=id_sb[0:qn, 0:qn],
                    rhs=bias_sb[0:qn, h, qt, :],
                    start=False, stop=True,
                )
                negm = stats.tile([128, 1], f32, tag="negm")
                nc.vector.tensor_reduce(
                    out=negm[0:qn, :], in_=s_ps[0:qn, :],
                    axis=mybir.AxisListType.X, op=ALU.max, negate=True,
                )
                p_sb = work.tile([128, N], f32, tag="p")
                rsum = stats.tile([128, 1], f32, tag="rsum")
                nc.scalar.activation(
                    out=p_sb[0:qn, :],
                    in_=s_ps[0:qn, :],
                    func=ACTF.Exp,
                    bias=negm[0:qn, :],
                    scale=1.0,
                    accum_out=rsum[0:qn, :],
                )
                rcp = stats.tile([128, 1], f32, tag="rcp")
                nc.vector.reciprocal(rcp[0:qn, :], rsum[0:qn, :])
                nc.vector.tensor_scalar_mul(
                    pn[0:qn, qt, :], p_sb[0:qn, :], rcp[0:qn, :]
                )

            # transpose Pn -> PnT (4 PE blocks, q contiguous per k-tile)
            pnT = work.tile([128, 2, N], bf16, tag="pnT", name="pnT")
            tr = tr_psum.tile([128, 512], bf16, tag="tr", name="tr")
            for kt in range(2):
                kn = qt_sizes[kt]
                for qt in range(2):
                    qn = qt_sizes[qt]
                    blk = tr[0:kn, kt * 256 + qt * 128:
                             kt * 256 + qt * 128 + qn]
                    nc.tensor.transpose(
                        blk,
                        in_=pn[0:qn, qt, kt * 128:kt * 128 + kn],
                        identity=id_sb[0:qn, 0:qn],
                    )
                src = tr[0:kn, kt * 256:kt * 256 + N]
                dst = pnT[0:kn, kt, :]
                nc.vector.tensor_copy(dst, src)

            # PV: outT[d, q] accumulated over k-tiles
            o_ps = o_psum.tile([64, N], f32, tag="o", name="o_ps")
            for kt in range(2):
                kn = qt_sizes[kt]
                nc.tensor.matmul(
                    o_ps[:, :],
                    lhsT=v_sb[0:kn, b, kt, h * 64:(h + 1) * 64],
                    rhs=pnT[0:kn, kt, :],
                    start=(kt == 0), stop=(kt == 1),
                )
            dst = attT_sb[po:po + 64, mq, b * N:(b + 1) * N]
            nc.scalar.copy(out=dst, in_=o_ps[:, :])

        def emit_proj(mt):
            rows = mt_sizes[mt]
            t0 = mt * 128
            for n2 in range(2):
                ps = mm_psum.tile([128, 512], f32, tag="mm", name="ps")
                for kc in range(KC):
                    nc.tensor.matmul(
                        ps[0:rows, 0:384],
                        lhsT=attT_sb[:, kc, t0:t0 + rows],
                        rhs=wp_sb[:, kc, n2 * 384:(n2 + 1) * 384],
                        start=(kc == 0), stop=(kc == KC - 1),
                    )
                yst = work.tile([128, 384], f32, tag="yst")
                nc.scalar.copy(out=yst[0:rows, :], in_=ps[0:rows, 0:384])
                nc.sync.dma_start(
                    out=y[t0:t0 + rows, n2 * 384:(n2 + 1) * 384],
                    in_=yst[0:rows, :],
                )

        # ---- emission: b-major; qk chunk-pairs stream in during b0,
        # v-proj just-in-time per batch, proj chunks as batches complete ----
        proj_ptr = [0]

        def emit_proj_upto(limit):
            while proj_ptr[0] < limit:
                emit_proj(proj_ptr[0])
                proj_ptr[0] += 1

        if probe >= 1:
            for b in range(bl):
                for hp in range(HEADS // 2):
                    if b == 0:
                        emit_qkproj(hp, 2 * hp)
                        emit_qkproj(KC + hp, 2 * hp + 1)
                    if hp == 0:
                        emit_vproj(b)
                    emit_attention(b, 2 * hp)
                    emit_attention(b, 2 * hp + 1)
                emit_proj_upto(((b + 1) * N) // 128)
            emit_proj_upto(len(mt_sizes))
        else:
            for mi, m in enumerate(range(2 * KC)):
                emit_qkproj(m, mi)
            for b in range(bl):
                emit_vproj(b)
            nc.vector.memset(attT_sb[:, :, :], 0.0)
            for mt in range(len(mt_sizes)):
                emit_proj(mt)

    nc.compile()
    return nc


def _prep_shared(w_qkv, w_proj, rel_pos, rel_pos_index):
    """Host-side input prep shared across cores (weights / bias / identity)."""
    w_qkv = np.asarray(w_qkv, dtype=np.float32)
    w_proj = np.asarray(w_proj, dtype=np.float32)
    rel_pos = np.asarray(rel_pos, dtype=np.float32)
    rel_pos_index = np.asarray(rel_pos_index)

    wqk = w_qkv[:2 * DIM].copy()
    wqk[:DIM] *= SCALE  # fold attention scale into Wq
    wqkT = np.ascontiguousarray(wqk.T).astype(BF16)
    wvT = np.ascontiguousarray(w_qkv[2 * DIM:].T).astype(BF16)
    wpT = np.ascontiguousarray(w_proj.T).astype(BF16)

    bias_full = np.zeros((HEADS, N, N), dtype=np.float32)
    bias_full[:, 1:, 1:] = rel_pos[:, rel_pos_index]
    bias_out = bias_full if BIAS_F32 else bias_full.astype(BF16)

    ident = np.eye(128, dtype=BF16)
    return {"wqkT": wqkT, "wvT": wvT, "wpT": wpT, "bias": bias_out, "ident": ident}


def _prep_core(x, core, bl=BL):
    """Per-core xT: [DIM, bl*N] bf16."""
    xc = np.asarray(x[core * bl:(core + 1) * bl], dtype=np.float32)
    xT = np.ascontiguousarray(xc.reshape(bl * N, DIM).T).astype(BF16)
    return xT


def kernel(x, w_qkv, w_proj, b_proj, rel_pos, rel_pos_index):
    from concourse.bass_utils import run_bass_kernel_spmd

    if "nc" not in _CACHE:
        _CACHE["nc"] = _build(BL)
    nc = _CACHE["nc"]

    shared = _prep_shared(w_qkv, w_proj, rel_pos, rel_pos_index)
    in_maps = []
    for core in range(NCORES):
        m = dict(shared)
        m["xT"] = _prep_core(x, core)
        in_maps.append(m)

    res = run_bass_kernel_spmd(nc, in_maps, core_ids=list(range(NCORES)))
    b_proj = np.asarray(b_proj, dtype=np.float32)
    y = np.concatenate(
        [r["y"].reshape(BL, N, DIM) for r in res.results], axis=0
    ).astype(np.float32)
    return y + b_proj[None, None, :]


# revision 39
# speedup vs baseline: 1.3611x; 1.0749x over previous
"""Trainium2 Bass kernel for AttentionWithRelPos.

Reference computation (fp32):
    qkv = x @ w_qkv.T                      # [B, N, 3C]
    q, k, v = split/reshape                # [B, H, N, HD]
    attn = softmax(q @ k.T * scale + bias) # bias gathered from rel_pos
    out  = (attn @ v).merge_heads @ w_proj.T + b_proj

Sharding: data-parallel over batch across 8 NeuronCores (8 batches/core).
All matmuls in bf16 with fp32 PSUM accumulation. Softmax is max-subtracted
(numerically safe for any input scale).

Per-core device pipeline (all feature-major / transposed layouts chosen so
no device-side transposes are needed except the softmax matrix itself):
  1. qkT = WqkT.T-stationary @ xT            -> [1536, 1576]   (q rows scaled)
  2. v   = xT-stationary @ WvT               -> [1576, 768]  (per-batch k-tiles)
  3. per (b, h):  S = qT.T @ kT  (q on partitions, k free)
     t = S + bias ; m = rowmax(t)  (fused DVE tensor_tensor_reduce)
     P = exp(t - m), rowsum via ACT accum_out ; r = 1/rowsum
     Pn = P * r (normalized, bf16)
     PnT = PE-transpose(Pn)  (4 blocks of <=128x128)
     outT = v-slice.T-stationary @ PnT       -> [64, 197] = attn-out head rows
  4. y = attT.T-stationary @ WpT             -> [1576, 768], PSUM -> DRAM
Host adds b_proj and re-assembles [64, 197, 768].
"""

import sys

if "/opt/trn_rl_repo" not in sys.path:
    sys.path.insert(0, "/opt/trn_rl_repo")

import numpy as np
import ml_dtypes

BF16 = ml_dtypes.bfloat16

B, DIM, HEADS, N = 64, 768, 12, 197
HD = DIM // HEADS  # 64
SCALE = HD ** -0.5
NCORES = 8
BL = B // NCORES  # 8 batches per core
KC = DIM // 128  # 6 contraction chunks

_CACHE = {}
BIAS_F32 = False
USE_TTR = False


def _build(bl=BL, probe=4, bias_f32=False):
    """Build + compile the per-core Bass program. Returns the compiled nc.

    probe: debug level — 0 skips attention; 1 up to S+ttr; 2 +exp/pn;
    3 +transposes; 4 full.
    """
    import concourse.bacc as bacc
    import concourse.bass as bass
    import concourse.tile as tile
    from concourse import mybir
    from contextlib import ExitStack

    sub = ""
    if isinstance(probe, str):
        probe, sub = 1, probe

    f32 = mybir.dt.float32
    bf16 = mybir.dt.bfloat16
    ALU = mybir.AluOpType
    ACTF = mybir.ActivationFunctionType

    tok = bl * N

    nc = bacc.Bacc("TRN2", target_bir_lowering=False, debug=False,
                   enable_asserts=False, num_devices=NCORES)

    xT = nc.dram_tensor("xT", (DIM, tok), bf16, kind="ExternalInput").ap()
    wqkT = nc.dram_tensor("wqkT", (DIM, 2 * DIM), bf16, kind="ExternalInput").ap()
    wvT = nc.dram_tensor("wvT", (DIM, DIM), bf16, kind="ExternalInput").ap()
    wpT = nc.dram_tensor("wpT", (DIM, DIM), bf16, kind="ExternalInput").ap()
    bias = nc.dram_tensor("bias", (HEADS, N, N), f32 if bias_f32 else bf16,
                          kind="ExternalInput").ap()
    ident = nc.dram_tensor("ident", (128, 128), bf16, kind="ExternalInput").ap()
    y = nc.dram_tensor("y", (tok, DIM), f32, kind="ExternalOutput").ap()

    # token-chunking for matmul moving dims
    NCH = 4 if tok % 4 == 0 else 1   # qk-proj rhs chunks
    CH = tok // NCH                  # 394 for bl=8
    assert CH <= 512
    # proj m-tiles (dense 128-token chunks)
    mt_sizes = [128] * (tok // 128) + ([tok % 128] if tok % 128 else [])

    with ExitStack() as ctx:
        tc = ctx.enter_context(tile.TileContext(nc))
        singles = ctx.enter_context(tc.tile_pool(name="singles", bufs=1))
        mm_psum = ctx.enter_context(tc.tile_pool(name="mm_psum", bufs=2, space="PSUM"))
        s_psum = ctx.enter_context(tc.tile_pool(name="s_psum", bufs=4, space="PSUM"))
        tr_psum = ctx.enter_context(tc.tile_pool(name="tr_psum", bufs=1, space="PSUM"))
        o_psum = ctx.enter_context(tc.tile_pool(name="o_psum", bufs=1, space="PSUM"))
        work = ctx.enter_context(tc.tile_pool(name="work", bufs=5))
        stats = ctx.enter_context(tc.tile_pool(name="stats", bufs=12))

        # ---- persistent SBUF tensors ----
        xT_sb = singles.tile([128, KC, tok], bf16)
        wqk_sb = singles.tile([128, KC, 2 * DIM], bf16)
        wv_sb = singles.tile([128, KC, DIM], bf16)
        wp_sb = singles.tile([128, KC, DIM], bf16)
        bias_sb = singles.tile([128, HEADS, 2, N], f32 if bias_f32 else bf16)
        id_sb = singles.tile([128, 128], bf16)
        qkT_sb = singles.tile([128, 2 * KC, tok], bf16)
        v_sb = singles.tile([128, bl, 2, DIM], bf16)
        attT_sb = singles.tile([128, KC, tok], bf16)

        # ---- input DMAs ----
        for kc in range(KC):
            nc.sync.dma_start(out=xT_sb[:, kc, :], in_=xT[kc * 128:(kc + 1) * 128, :])
            nc.sync.dma_start(out=wqk_sb[:, kc, :], in_=wqkT[kc * 128:(kc + 1) * 128, :])
            nc.sync.dma_start(out=wv_sb[:, kc, :], in_=wvT[kc * 128:(kc + 1) * 128, :])
            nc.sync.dma_start(out=wp_sb[:, kc, :], in_=wpT[kc * 128:(kc + 1) * 128, :])
        nc.sync.dma_start(out=id_sb[:, :], in_=ident[:, :])
        for h in range(HEADS):
            nc.sync.dma_start(out=bias_sb[:, h, 0, :], in_=bias[h, 0:128, :])
            nc.sync.dma_start(out=bias_sb[0:N - 128, h, 1, :], in_=bias[h, 128:N, :])

        qt_sizes = [128, N - 128]

        def emit_qkproj(m, mi):
            for n in range(NCH):
                ps = mm_psum.tile([128, 512], f32, tag="mm", name="ps")
                for kc in range(KC):
                    nc.tensor.matmul(
                        ps[:, 0:CH],
                        lhsT=wqk_sb[:, kc, m * 128:(m + 1) * 128],
                        rhs=xT_sb[:, kc, n * CH:(n + 1) * CH],
                        start=(kc == 0), stop=(kc == KC - 1),
                    )
                dst = qkT_sb[:, m, n * CH:(n + 1) * CH]
                nc.scalar.copy(out=dst, in_=ps[:, 0:CH])

        def emit_vproj(b):
            for kt in range(2):
                rows = 128 if kt == 0 else N - 128
                t0 = b * N + kt * 128
                for n2 in range(2):
                    ps = mm_psum.tile([128, 512], f32, tag="mm", name="ps")
                    for kc in range(KC):
                        nc.tensor.matmul(
                            ps[0:rows, 0:384],
                            lhsT=xT_sb[:, kc, t0:t0 + rows],
                            rhs=wv_sb[:, kc, n2 * 384:(n2 + 1) * 384],
                            start=(kc == 0), stop=(kc == KC - 1),
                        )
                    dst = v_sb[0:rows, b, kt, n2 * 384:(n2 + 1) * 384]
                    nc.vector.tensor_copy(dst, ps[0:rows, 0:384])

        def emit_attention(b, h):
            mq = h // 2
            mk = KC + h // 2
            po = (h % 2) * 64
            qT = qkT_sb[po:po + 64, mq, b * N:(b + 1) * N]
            kT = qkT_sb[po:po + 64, mk, b * N:(b + 1) * N]

            pn = work.tile([128, 2, N], bf16, tag="pn", name="pn")
            for qt in range(2):
                qn = qt_sizes[qt]
                s_ps = s_psum.tile([128, N], f32, tag="s", name="s_ps")
                # S = q.k^T; second matmul accumulates the rel-pos bias via
                # an identity-block stationary (bias rows are partition-major
                # in bias_sb)
                nc.tensor.matmul(
                    s_ps[0:qn, :],
                    lhsT=qT[:, qt * 128:qt * 128 + qn],
                    rhs=kT,
                    start=True, stop=False,
                )
                nc.tensor.matmul(
                    s_ps[0:qn, :],
                    lhsT=id_sb[0:qn, 0:qn],
                    rhs=bias_sb[0:qn, h, qt, :],
                    start=False, stop=True,
                )
                negm = stats.tile([128, 1], f32, tag="negm")
                nc.vector.tensor_reduce(
                    out=negm[0:qn, :], in_=s_ps[0:qn, :],
                    axis=mybir.AxisListType.X, op=ALU.max, negate=True,
                )
                p_sb = work.tile([128, N], f32, tag="p")
                rsum = stats.tile([128, 1], f32, tag="rsum")
                nc.scalar.activation(
                    out=p_sb[0:qn, :],
                    in_=s_ps[0:qn, :],
                    func=ACTF.Exp,
                    bias=negm[0:qn, :],
                    scale=1.0,
                    accum_out=rsum[0:qn, :],
                )
                rcp = stats.tile([128, 1], f32, tag="rcp")
                nc.vector.reciprocal(rcp[0:qn, :], rsum[0:qn, :])
                nc.vector.tensor_scalar_mul(
                    pn[0:qn, qt, :], p_sb[0:qn, :], rcp[0:qn, :]
                )

            # transpose Pn -> PnT (4 PE blocks, q contiguous per k-tile)
            pnT = work.tile([128, 2, N], bf16, tag="pnT", name="pnT")
            tr = tr_psum.tile([128, 512], bf16, tag="tr", name="tr")
            for kt in range(2):
                kn = qt_sizes[kt]
                for qt in range(2):
                    qn = qt_sizes[qt]
                    blk = tr[0:kn, kt * 256 + qt * 128:
                             kt * 256 + qt * 128 + qn]
                    nc.tensor.transpose(
                        blk,
                        in_=pn[0:qn, qt, kt * 128:kt * 128 + kn],
                        identity=id_sb[0:qn, 0:qn],
                    )
                src = tr[0:kn, kt * 256:kt * 256 + N]
                dst = pnT[0:kn, kt, :]
                nc.vector.tensor_copy(dst, src)

            # PV: outT[d, q] accumulated over k-tiles
            o_ps = o_psum.tile([64, N], f32, tag="o", name="o_ps")
            for kt in range(2):
                kn = qt_sizes[kt]
                nc.tensor.matmul(
                    o_ps[:, :],
                    lhsT=v_sb[0:kn, b, kt, h * 64:(h + 1) * 64],
                    rhs=pnT[0:kn, kt, :],
                    start=(kt == 0), stop=(kt == 1),
                )
            dst = attT_sb[po:po + 64, mq, b * N:(b + 1) * N]
            nc.scalar.copy(out=dst, in_=o_ps[:, :])

        def emit_proj(mt):
            rows = mt_sizes[mt]
            t0 = mt * 128
            for n2 in range(2):
                ps = mm_psum.tile([128, 512], f32, tag="mm", name="ps")
                for kc in range(KC):
                    nc.tensor.matmul(
                        ps[0:rows, 0:384],
                        lhsT=attT_sb[:, kc, t0:t0 + rows],
                        rhs=wp_sb[:, kc, n2 * 384:(n2 + 1) * 384],
                        start=(kc == 0), stop=(kc == KC - 1),
                    )
                yst = work.tile([128, 384], f32, tag="yst")
                nc.scalar.copy(out=yst[0:rows, :], in_=ps[0:rows, 0:384])
                nc.sync.dma_start(
                    out=y[t0:t0 + rows, n2 * 384:(n2 + 1) * 384],
                    in_=yst[0:rows, :],
                )

        # ---- emission: b-major; qk chunk-pairs stream in during b0,
        # v-proj just-in-time per batch, proj chunks as batches complete ----
        proj_ptr = [0]

        def emit_proj_upto(limit):
            while proj_ptr[0] < limit:
                emit_proj(proj_ptr[0])
                proj_ptr[0] += 1

        if probe >= 1:
            NHP = HEADS // 2
            for w in range(bl + NHP - 1):
                if w < NHP:
                    emit_qkproj(w, 2 * w)
                    emit_qkproj(KC + w, 2 * w + 1)
                for b in range(bl):
                    hp = w - b
                    if 0 <= hp < NHP:
                        if hp == 0:
                            emit_vproj(b)
                        emit_attention(b, 2 * hp)
                        emit_attention(b, 2 * hp + 1)
                if w >= NHP - 1:
                    emit_proj_upto(((w - NHP + 2) * N) // 128)
            emit_proj_upto(len(mt_sizes))
        else:
            for mi, m in enumerate(range(2 * KC)):
                emit_qkproj(m, mi)
            for b in range(bl):
                emit_vproj(b)
            nc.vector.memset(attT_sb[:, :, :], 0.0)
            for mt in range(len(mt_sizes)):
                emit_proj(mt)

    nc.compile()
    return nc


def _prep_shared(w_qkv, w_proj, rel_pos, rel_pos_index):
    """Host-side input prep shared across cores (weights / bias / identity)."""
    w_qkv = np.asarray(w_qkv, dtype=np.float32)
    w_proj = np.asarray(w_proj, dtype=np.float32)
    rel_pos = np.asarray(rel_pos, dtype=np.float32)
    rel_pos_index = np.asarray(rel_pos_index)

    wqk = w_qkv[:2 * DIM].copy()
    wqk[:DIM] *= SCALE  # fold attention scale into Wq
    wqkT = np.ascontiguousarray(wqk.T).astype(BF16)
    wvT = np.ascontiguousarray(w_qkv[2 * DIM:].T).astype(BF16)
    wpT = np.ascontiguousarray(w_proj.T).astype(BF16)

    bias_full = np.zeros((HEADS, N, N), dtype=np.float32)
    bias_full[:, 1:, 1:] = rel_pos[:, rel_pos_index]
    bias_out = bias_full if BIAS_F32 else bias_full.astype(BF16)

    ident = np.eye(128, dtype=BF16)
    return {"wqkT": wqkT, "wvT": wvT, "wpT": wpT, "bias": bias_out, "ident": ident}


def _prep_core(x, core, bl=BL):
    """Per-core xT: [DIM, bl*N] bf16."""
    xc = np.asarray(x[core * bl:(core + 1) * bl], dtype=np.float32)
    xT = np.ascontiguousarray(xc.reshape(bl * N, DIM).T).astype(BF16)
    return xT


def kernel(x, w_qkv, w_proj, b_proj, rel_pos, rel_pos_index):
    from concourse.bass_utils import run_bass_kernel_spmd

    x = np.asarray(x, dtype=np.float32)
    w_qkv = np.asarray(w_qkv, dtype=np.float32)
    w_proj = np.asarray(w_proj, dtype=np.float32)
    b_proj = np.asarray(b_proj, dtype=np.float32)
    rel_pos = np.asarray(rel_pos, dtype=np.float32)
    rel_pos_index = np.asarray(rel_pos_index)

    if "nc" not in _CACHE:
        _CACHE["nc"] = _build(BL)
    nc = _CACHE["nc"]

    shared = _prep_shared(w_qkv, w_proj, rel_pos, rel_pos_index)
    in_maps = []
    for core in range(NCORES):
        m = dict(shared)
        m["xT"] = _prep_core(x, core)
        in_maps.append(m)

    res = run_bass_kernel_spmd(nc, in_maps, core_ids=list(range(NCORES)))
    b_proj = np.asarray(b_proj, dtype=np.float32)
    y = np.concatenate(
        [r["y"].reshape(BL, N, DIM) for r in res.results], axis=0
    ).astype(np.float32)
    return y + b_proj[None, None, :]


# revision 40
# speedup vs baseline: 51.9835x; 38.1923x over previous
"""Trainium2 Bass kernel for AttentionWithRelPos.

Reference computation (fp32):
    qkv = x @ w_qkv.T                      # [B, N, 3C]
    q, k, v = split/reshape                # [B, H, N, HD]
    attn = softmax(q @ k.T * scale + bias) # bias gathered from rel_pos
    out  = (attn @ v).merge_heads @ w_proj.T + b_proj

Sharding: data-parallel over batch across 8 NeuronCores (8 batches/core).
All matmuls in bf16 with fp32 PSUM accumulation. Softmax is max-subtracted
(numerically safe for any input scale).

Per-core device pipeline (all feature-major / transposed layouts chosen so
no device-side transposes are needed except the softmax matrix itself):
  1. qkT = WqkT.T-stationary @ xT            -> [1536, 1576]   (q rows scaled)
  2. v   = xT-stationary @ WvT               -> [1576, 768]  (per-batch k-tiles)
  3. per (b, h):  S = qT.T @ kT  (q on partitions, k free), then the rel-pos
     bias is ACCUMULATED INTO THE SAME PSUM TILE by a second matmul with an
     identity-block stationary against the partition-major bias table (frees
     a whole DVE pass).
     m = rowmax (DVE, negated) ; P = exp(S+bias-m) with rowsum via ACT
     accum_out ; r = 1/rowsum (DVE) ; Pn = P*r -> bf16
     PnT = PE-transpose(Pn)  (4 blocks of <=128x128 into one PSUM bank,
     evacuated as 2 contiguous copies)
     outT = v-slice.T-stationary @ PnT       -> [64, 197] = attn-out head rows
  4. y = attT.T-stationary @ WpT             -> [1576, 768] -> DRAM
Emission is diagonal-wave interleaved (qk-proj chunk-pairs, per-batch v-proj,
attention, and trailing proj chunks all overlap; ~90% DVE/ACT occupancy in
steady state per the cost model).
Host adds b_proj and re-assembles [64, 197, 768].
"""

import sys

if "/opt/trn_rl_repo" not in sys.path:
    sys.path.insert(0, "/opt/trn_rl_repo")

import numpy as np
import ml_dtypes

BF16 = ml_dtypes.bfloat16

B, DIM, HEADS, N = 64, 768, 12, 197
HD = DIM // HEADS  # 64
SCALE = HD ** -0.5
NCORES = 8
BL = B // NCORES  # 8 batches per core
KC = DIM // 128  # 6 contraction chunks

_CACHE = {}
BIAS_F32 = False
USE_TTR = False


def _build(bl=BL, probe=4, bias_f32=False):
    """Build + compile the per-core Bass program. Returns the compiled nc.

    probe: debug level — 0 skips attention; 1 up to S+ttr; 2 +exp/pn;
    3 +transposes; 4 full.
    """
    import concourse.bacc as bacc
    import concourse.bass as bass
    import concourse.tile as tile
    from concourse import mybir
    from contextlib import ExitStack

    sub = ""
    if isinstance(probe, str):
        probe, sub = 1, probe

    f32 = mybir.dt.float32
    bf16 = mybir.dt.bfloat16
    ALU = mybir.AluOpType
    ACTF = mybir.ActivationFunctionType

    tok = bl * N

    nc = bacc.Bacc("TRN2", target_bir_lowering=False, debug=False,
                   enable_asserts=False, num_devices=NCORES)

    xT = nc.dram_tensor("xT", (DIM, tok), bf16, kind="ExternalInput").ap()
    wqkT = nc.dram_tensor("wqkT", (DIM, 2 * DIM), bf16, kind="ExternalInput").ap()
    wvT = nc.dram_tensor("wvT", (DIM, DIM), bf16, kind="ExternalInput").ap()
    wpT = nc.dram_tensor("wpT", (DIM, DIM), bf16, kind="ExternalInput").ap()
    bias = nc.dram_tensor("bias", (HEADS, N, N), f32 if bias_f32 else bf16,
                          kind="ExternalInput").ap()
    ident = nc.dram_tensor("ident", (128, 128), bf16, kind="ExternalInput").ap()
    y = nc.dram_tensor("y", (tok, DIM), f32, kind="ExternalOutput").ap()

    # token-chunking for matmul moving dims
    NCH = 4 if tok % 4 == 0 else 1   # qk-proj rhs chunks
    CH = tok // NCH                  # 394 for bl=8
    assert CH <= 512
    # proj m-tiles (dense 128-token chunks)
    mt_sizes = [128] * (tok // 128) + ([tok % 128] if tok % 128 else [])

    with ExitStack() as ctx:
        tc = ctx.enter_context(tile.TileContext(nc))
        singles = ctx.enter_context(tc.tile_pool(name="singles", bufs=1))
        mm_psum = ctx.enter_context(tc.tile_pool(name="mm_psum", bufs=2, space="PSUM"))
        s_psum = ctx.enter_context(tc.tile_pool(name="s_psum", bufs=4, space="PSUM"))
        tr_psum = ctx.enter_context(tc.tile_pool(name="tr_psum", bufs=1, space="PSUM"))
        o_psum = ctx.enter_context(tc.tile_pool(name="o_psum", bufs=1, space="PSUM"))
        work = ctx.enter_context(tc.tile_pool(name="work", bufs=5))
        stats = ctx.enter_context(tc.tile_pool(name="stats", bufs=12))

        # ---- persistent SBUF tensors ----
        xT_sb = singles.tile([128, KC, tok], bf16)
        wqk_sb = singles.tile([128, KC, 2 * DIM], bf16)
        wv_sb = singles.tile([128, KC, DIM], bf16)
        wp_sb = singles.tile([128, KC, DIM], bf16)
        bias_sb = singles.tile([128, HEADS, 2, N], f32 if bias_f32 else bf16)
        id_sb = singles.tile([128, 128], bf16)
        qkT_sb = singles.tile([128, 2 * KC, tok], bf16)
        v_sb = singles.tile([128, bl, 2, DIM], bf16)
        attT_sb = singles.tile([128, KC, tok], bf16)

        # ---- input DMAs ----
        for kc in range(KC):
            nc.sync.dma_start(out=xT_sb[:, kc, :], in_=xT[kc * 128:(kc + 1) * 128, :])
            nc.sync.dma_start(out=wqk_sb[:, kc, :], in_=wqkT[kc * 128:(kc + 1) * 128, :])
            nc.sync.dma_start(out=wv_sb[:, kc, :], in_=wvT[kc * 128:(kc + 1) * 128, :])
            nc.sync.dma_start(out=wp_sb[:, kc, :], in_=wpT[kc * 128:(kc + 1) * 128, :])
        nc.sync.dma_start(out=id_sb[:, :], in_=ident[:, :])
        for h in range(HEADS):
            nc.sync.dma_start(out=bias_sb[:, h, 0, :], in_=bias[h, 0:128, :])
            nc.sync.dma_start(out=bias_sb[0:N - 128, h, 1, :], in_=bias[h, 128:N, :])

        qt_sizes = [128, N - 128]

        def emit_qkproj(m, mi):
            for n in range(NCH):
                ps = mm_psum.tile([128, 512], f32, tag="mm", name="ps")
                for kc in range(KC):
                    nc.tensor.matmul(
                        ps[:, 0:CH],
                        lhsT=wqk_sb[:, kc, m * 128:(m + 1) * 128],
                        rhs=xT_sb[:, kc, n * CH:(n + 1) * CH],
                        start=(kc == 0), stop=(kc == KC - 1),
                    )
                dst = qkT_sb[:, m, n * CH:(n + 1) * CH]
                nc.scalar.copy(out=dst, in_=ps[:, 0:CH])

        def emit_vproj(b):
            for kt in range(2):
                rows = 128 if kt == 0 else N - 128
                t0 = b * N + kt * 128
                for n2 in range(2):
                    ps = mm_psum.tile([128, 512], f32, tag="mm", name="ps")
                    for kc in range(KC):
                        nc.tensor.matmul(
                            ps[0:rows, 0:384],
                            lhsT=xT_sb[:, kc, t0:t0 + rows],
                            rhs=wv_sb[:, kc, n2 * 384:(n2 + 1) * 384],
                            start=(kc == 0), stop=(kc == KC - 1),
                        )
                    dst = v_sb[0:rows, b, kt, n2 * 384:(n2 + 1) * 384]
                    nc.vector.tensor_copy(dst, ps[0:rows, 0:384])

        def emit_attention(b, h):
            mq = h // 2
            mk = KC + h // 2
            po = (h % 2) * 64
            qT = qkT_sb[po:po + 64, mq, b * N:(b + 1) * N]
            kT = qkT_sb[po:po + 64, mk, b * N:(b + 1) * N]

            pn = work.tile([128, 2, N], bf16, tag="pn", name="pn")
            for qt in range(2):
                qn = qt_sizes[qt]
                s_ps = s_psum.tile([128, N], f32, tag="s", name="s_ps")
                # S = q.k^T; second matmul accumulates the rel-pos bias via
                # an identity-block stationary (bias rows are partition-major
                # in bias_sb)
                nc.tensor.matmul(
                    s_ps[0:qn, :],
                    lhsT=qT[:, qt * 128:qt * 128 + qn],
                    rhs=kT,
                    start=True, stop=False,
                )
                nc.tensor.matmul(
                    s_ps[0:qn, :],
                    lhsT=id_sb[0:qn, 0:qn],
                    rhs=bias_sb[0:qn, h, qt, :],
                    start=False, stop=True,
                )
                negm = stats.tile([128, 1], f32, tag="negm")
                nc.vector.tensor_reduce(
                    out=negm[0:qn, :], in_=s_ps[0:qn, :],
                    axis=mybir.AxisListType.X, op=ALU.max, negate=True,
                )
                p_sb = work.tile([128, N], f32, tag="p")
                rsum = stats.tile([128, 1], f32, tag="rsum")
                nc.scalar.activation(
                    out=p_sb[0:qn, :],
                    in_=s_ps[0:qn, :],
                    func=ACTF.Exp,
                    bias=negm[0:qn, :],
                    scale=1.0,
                    accum_out=rsum[0:qn, :],
                )
                rcp = stats.tile([128, 1], f32, tag="rcp")
                nc.vector.reciprocal(rcp[0:qn, :], rsum[0:qn, :])
                nc.vector.tensor_scalar_mul(
                    pn[0:qn, qt, :], p_sb[0:qn, :], rcp[0:qn, :]
                )

            # transpose Pn -> PnT (4 PE blocks, q contiguous per k-tile)
            pnT = work.tile([128, 2, N], bf16, tag="pnT", name="pnT")
            tr = tr_psum.tile([128, 512], bf16, tag="tr", name="tr")
            for kt in range(2):
                kn = qt_sizes[kt]
                for qt in range(2):
                    qn = qt_sizes[qt]
                    blk = tr[0:kn, kt * 256 + qt * 128:
                             kt * 256 + qt * 128 + qn]
                    nc.tensor.transpose(
                        blk,
                        in_=pn[0:qn, qt, kt * 128:kt * 128 + kn],
                        identity=id_sb[0:qn, 0:qn],
                    )
                src = tr[0:kn, kt * 256:kt * 256 + N]
                dst = pnT[0:kn, kt, :]
                nc.vector.tensor_copy(dst, src)

            # PV: outT[d, q] accumulated over k-tiles
            o_ps = o_psum.tile([64, N], f32, tag="o", name="o_ps")
            for kt in range(2):
                kn = qt_sizes[kt]
                nc.tensor.matmul(
                    o_ps[:, :],
                    lhsT=v_sb[0:kn, b, kt, h * 64:(h + 1) * 64],
                    rhs=pnT[0:kn, kt, :],
                    start=(kt == 0), stop=(kt == 1),
                )
            dst = attT_sb[po:po + 64, mq, b * N:(b + 1) * N]
            nc.scalar.copy(out=dst, in_=o_ps[:, :])

        def emit_proj(mt):
            rows = mt_sizes[mt]
            t0 = mt * 128
            for n2 in range(2):
                ps = mm_psum.tile([128, 512], f32, tag="mm", name="ps")
                for kc in range(KC):
                    nc.tensor.matmul(
                        ps[0:rows, 0:384],
                        lhsT=attT_sb[:, kc, t0:t0 + rows],
                        rhs=wp_sb[:, kc, n2 * 384:(n2 + 1) * 384],
                        start=(kc == 0), stop=(kc == KC - 1),
                    )
                yst = work.tile([128, 384], f32, tag="yst")
                nc.scalar.copy(out=yst[0:rows, :], in_=ps[0:rows, 0:384])
                nc.sync.dma_start(
                    out=y[t0:t0 + rows, n2 * 384:(n2 + 1) * 384],
                    in_=yst[0:rows, :],
                )

        # ---- emission: b-major; qk chunk-pairs stream in during b0,
        # v-proj just-in-time per batch, proj chunks as batches complete ----
        proj_ptr = [0]

        def emit_proj_upto(limit):
            while proj_ptr[0] < limit:
                emit_proj(proj_ptr[0])
                proj_ptr[0] += 1

        if probe >= 1:
            NHP = HEADS // 2
            for w in range(bl + NHP - 1):
                if w < NHP:
                    emit_qkproj(w, 2 * w)
                    emit_qkproj(KC + w, 2 * w + 1)
                for b in range(bl):
                    hp = w - b
                    if 0 <= hp < NHP:
                        if hp == 0:
                            emit_vproj(b)
                        emit_attention(b, 2 * hp)
                        emit_attention(b, 2 * hp + 1)
                if w >= NHP - 1:
                    emit_proj_upto(((w - NHP + 2) * N) // 128)
            emit_proj_upto(len(mt_sizes))
        else:
            for mi, m in enumerate(range(2 * KC)):
                emit_qkproj(m, mi)
            for b in range(bl):
                emit_vproj(b)
            nc.vector.memset(attT_sb[:, :, :], 0.0)
            for mt in range(len(mt_sizes)):
                emit_proj(mt)

    nc.compile()
    return nc


def _prep_shared(w_qkv, w_proj, rel_pos, rel_pos_index):
    """Host-side input prep shared across cores (weights / bias / identity)."""
    w_qkv = np.asarray(w_qkv, dtype=np.float32)
    w_proj = np.asarray(w_proj, dtype=np.float32)
    rel_pos = np.asarray(rel_pos, dtype=np.float32)
    rel_pos_index = np.asarray(rel_pos_index)

    wqk = w_qkv[:2 * DIM].copy()
    wqk[:DIM] *= SCALE  # fold attention scale into Wq
    wqkT = np.ascontiguousarray(wqk.T).astype(BF16)
    wvT = np.ascontiguousarray(w_qkv[2 * DIM:].T).astype(BF16)
    wpT = np.ascontiguousarray(w_proj.T).astype(BF16)

    bias_full = np.zeros((HEADS, N, N), dtype=np.float32)
    bias_full[:, 1:, 1:] = rel_pos[:, rel_pos_index]
    bias_out = bias_full if BIAS_F32 else bias_full.astype(BF16)

    ident = np.eye(128, dtype=BF16)
    return {"wqkT": wqkT, "wvT": wvT, "wpT": wpT, "bias": bias_out, "ident": ident}


def _prep_core(x, core, bl=BL):
    """Per-core xT: [DIM, bl*N] bf16."""
    xc = np.asarray(x[core * bl:(core + 1) * bl], dtype=np.float32)
    xT = np.ascontiguousarray(xc.reshape(bl * N, DIM).T).astype(BF16)
    return xT


def kernel(x, w_qkv, w_proj, b_proj, rel_pos, rel_pos_index):
    from concourse.bass_utils import run_bass_kernel_spmd

    x = np.asarray(x, dtype=np.float32)
    w_qkv = np.asarray(w_qkv, dtype=np.float32)
    w_proj = np.asarray(w_proj, dtype=np.float32)
    b_proj = np.asarray(b_proj, dtype=np.float32)
    rel_pos = np.asarray(rel_pos, dtype=np.float32)
    rel_pos_index = np.asarray(rel_pos_index)

    if "nc" not in _CACHE:
        _CACHE["nc"] = _build(BL)
    nc = _CACHE["nc"]

    shared = _prep_shared(w_qkv, w_proj, rel_pos, rel_pos_index)
    in_maps = []
    for core in range(NCORES):
        m = dict(shared)
        m["xT"] = _prep_core(x, core)
        in_maps.append(m)

    res = run_bass_kernel_spmd(nc, in_maps, core_ids=list(range(NCORES)))
    b_proj = np.asarray(b_proj, dtype=np.float32)
    y = np.concatenate(
        [r["y"].reshape(BL, N, DIM) for r in res.results], axis=0
    ).astype(np.float32)
    return y + b_proj[None, None, :]


# revision 44
# speedup vs baseline: 70.7005x; 1.3601x over previous
"""Trainium2 Bass kernel for AttentionWithRelPos.

Reference computation (fp32):
    qkv = x @ w_qkv.T                      # [B, N, 3C]
    q, k, v = split/reshape                # [B, H, N, HD]
    attn = softmax(q @ k.T * scale + bias) # bias gathered from rel_pos
    out  = (attn @ v).merge_heads @ w_proj.T + b_proj

Sharding: data-parallel over batch across 8 NeuronCores (8 batches/core).
All matmuls in bf16 with fp32 PSUM accumulation. Softmax is max-subtracted
(numerically safe for any input scale).

Per-core device pipeline (all feature-major / transposed layouts chosen so
no device-side transposes are needed except the softmax matrix itself):
  1. qkT = WqkT.T-stationary @ xT            -> [1536, 1576]   (q rows scaled)
  2. v   = xT-stationary @ WvT               -> [1576, 768]  (per-batch k-tiles)
  3. per (b, h):  S = qT.T @ kT  (q on partitions, k free), then the rel-pos
     bias is ACCUMULATED INTO THE SAME PSUM TILE by a second matmul with an
     identity-block stationary against the partition-major bias table (frees
     a whole DVE pass).
     m = rowmax (DVE, negated) ; P = exp(S+bias-m) with rowsum via ACT
     accum_out ; r = 1/rowsum (DVE) ; Pn = P*r -> bf16
     PnT = PE-transpose(Pn)  (4 blocks of <=128x128 into one PSUM bank,
     evacuated as 2 contiguous copies)
     outT = v-slice.T-stationary @ PnT       -> [64, 197] = attn-out head rows
  4. y = attT.T-stationary @ WpT             -> [1576, 768] -> DRAM
Emission is diagonal-wave interleaved (qk-proj chunk-pairs, per-batch v-proj,
attention, and trailing proj chunks all overlap; ~90% DVE/ACT occupancy in
steady state per the cost model).
Host adds b_proj and re-assembles [64, 197, 768].
"""

import sys

if "/opt/trn_rl_repo" not in sys.path:
    sys.path.insert(0, "/opt/trn_rl_repo")

import numpy as np
import ml_dtypes

BF16 = ml_dtypes.bfloat16

B, DIM, HEADS, N = 64, 768, 12, 197
HD = DIM // HEADS  # 64
SCALE = HD ** -0.5
NCORES = 8
BL = B // NCORES  # 8 batches per core
KC = DIM // 128  # 6 contraction chunks

_CACHE = {}
BIAS_F32 = False
USE_TTR = False


def _build(bl=BL, probe=4, bias_f32=False):
    """Build + compile the per-core Bass program. Returns the compiled nc.

    probe: debug level — 0 skips attention; 1 up to S+ttr; 2 +exp/pn;
    3 +transposes; 4 full.
    """
    import concourse.bacc as bacc
    import concourse.bass as bass
    import concourse.tile as tile
    from concourse import mybir
    from contextlib import ExitStack

    sub = ""
    if isinstance(probe, str):
        probe, sub = 1, probe

    f32 = mybir.dt.float32
    bf16 = mybir.dt.bfloat16
    ALU = mybir.AluOpType
    ACTF = mybir.ActivationFunctionType

    tok = bl * N

    nc = bacc.Bacc("TRN2", target_bir_lowering=False, debug=False,
                   enable_asserts=False, num_devices=NCORES)

    xT = nc.dram_tensor("xT", (DIM, tok), bf16, kind="ExternalInput").ap()
    wqkT = nc.dram_tensor("wqkT", (DIM, 2 * DIM), bf16, kind="ExternalInput").ap()
    wvT = nc.dram_tensor("wvT", (DIM, DIM), bf16, kind="ExternalInput").ap()
    wpT = nc.dram_tensor("wpT", (DIM, DIM), bf16, kind="ExternalInput").ap()
    bias = nc.dram_tensor("bias", (HEADS, N, N), f32 if bias_f32 else bf16,
                          kind="ExternalInput").ap()
    ident = nc.dram_tensor("ident", (128, 128), bf16, kind="ExternalInput").ap()
    y = nc.dram_tensor("y", (tok, DIM), f32, kind="ExternalOutput").ap()

    # token-chunking for matmul moving dims
    NCH = 4 if tok % 4 == 0 else 1   # qk-proj rhs chunks
    CH = tok // NCH                  # 394 for bl=8
    assert CH <= 512
    # proj m-tiles (dense 128-token chunks)
    mt_sizes = [128] * (tok // 128) + ([tok % 128] if tok % 128 else [])

    with ExitStack() as ctx:
        tc = ctx.enter_context(tile.TileContext(nc))
        singles = ctx.enter_context(tc.tile_pool(name="singles", bufs=1))
        mm_psum = ctx.enter_context(tc.tile_pool(name="mm_psum", bufs=2, space="PSUM"))
        s_psum = ctx.enter_context(tc.tile_pool(name="s_psum", bufs=4, space="PSUM"))
        tr_psum = ctx.enter_context(tc.tile_pool(name="tr_psum", bufs=1, space="PSUM"))
        o_psum = ctx.enter_context(tc.tile_pool(name="o_psum", bufs=1, space="PSUM"))
        work = ctx.enter_context(tc.tile_pool(name="work", bufs=5))
        stats = ctx.enter_context(tc.tile_pool(name="stats", bufs=12))

        # ---- persistent SBUF tensors ----
        xT_sb = singles.tile([128, KC, tok], bf16)
        wqk_sb = singles.tile([128, KC, 2 * DIM], bf16)
        wv_sb = singles.tile([128, KC, DIM], bf16)
        wp_sb = singles.tile([128, KC, DIM], bf16)
        bias_sb = singles.tile([128, HEADS, 2, N], f32 if bias_f32 else bf16)
        id_sb = singles.tile([128, 128], bf16)
        qkT_sb = singles.tile([128, 2 * KC, tok], bf16)
        v_sb = singles.tile([128, bl, 2, DIM], bf16)
        attT_sb = singles.tile([128, KC, tok], bf16)

        # ---- input DMAs ----
        for kc in range(KC):
            nc.sync.dma_start(out=xT_sb[:, kc, :], in_=xT[kc * 128:(kc + 1) * 128, :])
            nc.sync.dma_start(out=wqk_sb[:, kc, :], in_=wqkT[kc * 128:(kc + 1) * 128, :])
            nc.sync.dma_start(out=wv_sb[:, kc, :], in_=wvT[kc * 128:(kc + 1) * 128, :])
            nc.sync.dma_start(out=wp_sb[:, kc, :], in_=wpT[kc * 128:(kc + 1) * 128, :])
        nc.sync.dma_start(out=id_sb[:, :], in_=ident[:, :])
        for h in range(HEADS):
            nc.sync.dma_start(out=bias_sb[:, h, 0, :], in_=bias[h, 0:128, :])
            nc.sync.dma_start(out=bias_sb[0:N - 128, h, 1, :], in_=bias[h, 128:N, :])

        qt_sizes = [128, N - 128]

        def emit_qkproj(m, mi):
            for n in range(NCH):
                ps = mm_psum.tile([128, 512], f32, tag="mm", name="ps")
                for kc in range(KC):
                    nc.tensor.matmul(
                        ps[:, 0:CH],
                        lhsT=wqk_sb[:, kc, m * 128:(m + 1) * 128],
                        rhs=xT_sb[:, kc, n * CH:(n + 1) * CH],
                        start=(kc == 0), stop=(kc == KC - 1),
                    )
                dst = qkT_sb[:, m, n * CH:(n + 1) * CH]
                nc.scalar.copy(out=dst, in_=ps[:, 0:CH])

        def emit_vproj(b):
            for kt in range(2):
                rows = 128 if kt == 0 else N - 128
                t0 = b * N + kt * 128
                for n2 in range(2):
                    ps = mm_psum.tile([128, 512], f32, tag="mm", name="ps")
                    for kc in range(KC):
                        nc.tensor.matmul(
                            ps[0:rows, 0:384],
                            lhsT=xT_sb[:, kc, t0:t0 + rows],
                            rhs=wv_sb[:, kc, n2 * 384:(n2 + 1) * 384],
                            start=(kc == 0), stop=(kc == KC - 1),
                        )
                    dst = v_sb[0:rows, b, kt, n2 * 384:(n2 + 1) * 384]
                    nc.vector.tensor_copy(dst, ps[0:rows, 0:384])

        def emit_attention(b, h):
            mq = h // 2
            mk = KC + h // 2
            po = (h % 2) * 64
            qT = qkT_sb[po:po + 64, mq, b * N:(b + 1) * N]
            kT = qkT_sb[po:po + 64, mk, b * N:(b + 1) * N]

            pn = work.tile([128, 2, N], bf16, tag="pn", name="pn")
            for qt in range(2):
                qn = qt_sizes[qt]
                s_ps = s_psum.tile([128, N], f32, tag="s", name="s_ps")
                # S = q.k^T; second matmul accumulates the rel-pos bias via
                # an identity-block stationary (bias rows are partition-major
                # in bias_sb)
                nc.tensor.matmul(
                    s_ps[0:qn, :],
                    lhsT=qT[:, qt * 128:qt * 128 + qn],
                    rhs=kT,
                    start=True, stop=False,
                )
                nc.tensor.matmul(
                    s_ps[0:qn, :],
                    lhsT=id_sb[0:qn, 0:qn],
                    rhs=bias_sb[0:qn, h, qt, :],
                    start=False, stop=True,
                )
                negm = stats.tile([128, 1], f32, tag="negm")
                nc.vector.tensor_reduce(
                    out=negm[0:qn, :], in_=s_ps[0:qn, :],
                    axis=mybir.AxisListType.X, op=ALU.max, negate=True,
                )
                p_sb = work.tile([128, N], f32, tag="p")
                rsum = stats.tile([128, 1], f32, tag="rsum")
                nc.scalar.activation(
                    out=p_sb[0:qn, :],
                    in_=s_ps[0:qn, :],
                    func=ACTF.Exp,
                    bias=negm[0:qn, :],
                    scale=1.0,
                    accum_out=rsum[0:qn, :],
                )
                rcp = stats.tile([128, 1], f32, tag="rcp")
                nc.vector.reciprocal(rcp[0:qn, :], rsum[0:qn, :])
                nc.gpsimd.tensor_scalar_mul(
                    pn[0:qn, qt, :], p_sb[0:qn, :], rcp[0:qn, :]
                )

            # transpose Pn -> PnT (4 PE blocks, q contiguous per k-tile)
            pnT = work.tile([128, 2, N], bf16, tag="pnT", name="pnT")
            tr = tr_psum.tile([128, 512], bf16, tag="tr", name="tr")
            for kt in range(2):
                kn = qt_sizes[kt]
                for qt in range(2):
                    qn = qt_sizes[qt]
                    blk = tr[0:kn, kt * 256 + qt * 128:
                             kt * 256 + qt * 128 + qn]
                    nc.tensor.transpose(
                        blk,
                        in_=pn[0:qn, qt, kt * 128:kt * 128 + kn],
                        identity=id_sb[0:qn, 0:qn],
                    )
                src = tr[0:kn, kt * 256:kt * 256 + N]
                dst = pnT[0:kn, kt, :]
                nc.vector.tensor_copy(dst, src)

            # PV: outT[d, q] accumulated over k-tiles
            o_ps = o_psum.tile([64, N], f32, tag="o", name="o_ps")
            for kt in range(2):
                kn = qt_sizes[kt]
                nc.tensor.matmul(
                    o_ps[:, :],
                    lhsT=v_sb[0:kn, b, kt, h * 64:(h + 1) * 64],
                    rhs=pnT[0:kn, kt, :],
                    start=(kt == 0), stop=(kt == 1),
                )
            dst = attT_sb[po:po + 64, mq, b * N:(b + 1) * N]
            if (b + h) % 2 == 0:
                nc.scalar.copy(out=dst, in_=o_ps[:, :])
            else:
                nc.vector.tensor_copy(dst, o_ps[:, :])

        def emit_proj(mt):
            rows = mt_sizes[mt]
            t0 = mt * 128
            for n2 in range(2):
                ps = mm_psum.tile([128, 512], f32, tag="mm", name="ps")
                for kc in range(KC):
                    nc.tensor.matmul(
                        ps[0:rows, 0:384],
                        lhsT=attT_sb[:, kc, t0:t0 + rows],
                        rhs=wp_sb[:, kc, n2 * 384:(n2 + 1) * 384],
                        start=(kc == 0), stop=(kc == KC - 1),
                    )
                yst = work.tile([128, 384], f32, tag="yst")
                nc.scalar.copy(out=yst[0:rows, :], in_=ps[0:rows, 0:384])
                nc.sync.dma_start(
                    out=y[t0:t0 + rows, n2 * 384:(n2 + 1) * 384],
                    in_=yst[0:rows, :],
                )

        # ---- emission: b-major; qk chunk-pairs stream in during b0,
        # v-proj just-in-time per batch, proj chunks as batches complete ----
        proj_ptr = [0]

        def emit_proj_upto(limit):
            while proj_ptr[0] < limit:
                emit_proj(proj_ptr[0])
                proj_ptr[0] += 1

        if probe >= 1:
            NHP = HEADS // 2
            for w in range(bl + NHP - 1):
                if w < NHP:
                    emit_qkproj(w, 2 * w)
                    emit_qkproj(KC + w, 2 * w + 1)
                for b in range(bl):
                    hp = w - b
                    if 0 <= hp < NHP:
                        if hp == 0:
                            emit_vproj(b)
                        emit_attention(b, 2 * hp)
                        emit_attention(b, 2 * hp + 1)
                if w >= NHP - 1:
                    emit_proj_upto(((w - NHP + 2) * N) // 128)
            emit_proj_upto(len(mt_sizes))
        else:
            for mi, m in enumerate(range(2 * KC)):
                emit_qkproj(m, mi)
            for b in range(bl):
                emit_vproj(b)
            nc.vector.memset(attT_sb[:, :, :], 0.0)
            for mt in range(len(mt_sizes)):
                emit_proj(mt)

    nc.compile()
    return nc


def _prep_shared(w_qkv, w_proj, rel_pos, rel_pos_index):
    """Host-side input prep shared across cores (weights / bias / identity)."""
    w_qkv = np.asarray(w_qkv, dtype=np.float32)
    w_proj = np.asarray(w_proj, dtype=np.float32)
    rel_pos = np.asarray(rel_pos, dtype=np.float32)
    rel_pos_index = np.asarray(rel_pos_index)

    wqk = w_qkv[:2 * DIM].copy()
    wqk[:DIM] *= SCALE  # fold attention scale into Wq
    wqkT = np.ascontiguousarray(wqk.T).astype(BF16)
    wvT = np.ascontiguousarray(w_qkv[2 * DIM:].T).astype(BF16)
    wpT = np.ascontiguousarray(w_proj.T).astype(BF16)

    bias_full = np.zeros((HEADS, N, N), dtype=np.float32)
    bias_full[:, 1:, 1:] = rel_pos[:, rel_pos_index]
    bias_out = bias_full if BIAS_F32 else bias_full.astype(BF16)

    ident = np.eye(128, dtype=BF16)
    return {"wqkT": wqkT, "wvT": wvT, "wpT": wpT, "bias": bias_out, "ident": ident}


def _prep_core(x, core, bl=BL):
    """Per-core xT: [DIM, bl*N] bf16."""
    xc = np.asarray(x[core * bl:(core + 1) * bl], dtype=np.float32)
    xT = np.ascontiguousarray(xc.reshape(bl * N, DIM).T).astype(BF16)
    return xT


def kernel(x, w_qkv, w_proj, b_proj, rel_pos, rel_pos_index):
    from concourse.bass_utils import run_bass_kernel_spmd

    x = np.asarray(x, dtype=np.float32)
    w_qkv = np.asarray(w_qkv, dtype=np.float32)
    w_proj = np.asarray(w_proj, dtype=np.float32)
    b_proj = np.asarray(b_proj, dtype=np.float32)
    rel_pos = np.asarray(rel_pos, dtype=np.float32)
    rel_pos_index = np.asarray(rel_pos_index)

    if "nc" not in _CACHE:
        _CACHE["nc"] = _build(BL)
    nc = _CACHE["nc"]

    shared = _prep_shared(w_qkv, w_proj, rel_pos, rel_pos_index)
    in_maps = []
    for core in range(NCORES):
        m = dict(shared)
        m["xT"] = _prep_core(x, core)
        in_maps.append(m)

    res = run_bass_kernel_spmd(nc, in_maps, core_ids=list(range(NCORES)))
    b_proj = np.asarray(b_proj, dtype=np.float32)
    y = np.concatenate(
        [r["y"].reshape(BL, N, DIM) for r in res.results], axis=0
    ).astype(np.float32)
    return y + b_proj[None, None, :]
